# revision 25
# baseline (speedup 1.0000x reference)
"""Causal self-attention (B=2, S=2048, H=1024, NH=16) on 8 TRN2 NeuronCores.

Sharding: core c handles batch b = c//4 and heads [4*(c%4), 4*(c%4)+4).
Tensor-parallel c_attn (column split); the output projection is computed
from the full head dimension on a 512-row output slice per core, after
per-chunk AllGathers inside each 4-core batch group redistribute O^T.
The per-rank gathered-block offset is selected with a data-driven register
offset (qoff) so the program stays SPMD-uniform.

All matmul operands are bf16 (PE runs 1 cycle/row vs 4 for fp32 HIGH),
with fp32 PSUM accumulation; biases and the final output stay fp32.

Per-core dataflow (all matmuls on the PE array):
  1. x[b] -> x^T via PE transposes, 4 per PSUM tile, one batched copy.
  2. QKV: Q^T,K^T [dk, S] and V [S, dk] for its 4 heads (Wq pre-scaled 1/8).
  3. Attention per head in "transposed space", j-outer h-inner:
     S^T[k,q] = K-block @ Q^T for groups of 2 k-blocks into a [128,1024]
     PSUM tile, one batched exp on ACT per group, batched causal corner
     mask on DVE, then out^T = [V|1].T @ A^T accumulated per chunk,
     software-pipelined so the PE never waits on the current group's exp.
     The appended ones-column yields softmax denominators in PSUM row 64;
     normalization reshapes that row to [128,4] via a DRAM bounce (a
     [1,512] DVE reciprocal costs ~8 cyc/element, the reshaped one ~200ns),
     reciprocals it, and broadcast-DMAs it back for one DVE multiply.
  4. Per-chunk AllGather of O^T (overlapped with later chunks' attention),
     then c_proj of the core's 512-row slice.
"""

import sys

sys.path.insert(0, "/opt/trn_rl_repo")

import numpy as np

import concourse.bass as bass
import concourse.mybir as mybir
import concourse.tile as tile
from concourse import bacc
from concourse.bass_utils import run_bass_kernel_spmd
from concourse.masks import make_identity

B, S, H, NH, DK = 2, 2048, 1024, 16, 64
NCORES = 8
HPC = 4            # heads per core
CW = HPC * DK      # 256 qkv columns per core
SLICE = S // 4     # 512 output rows per core
GROUPS = [[0, 1, 2, 3], [4, 5, 6, 7]]

F32 = mybir.dt.float32
BF16 = mybir.dt.bfloat16

KT = H // 128   # 8 contraction tiles over H
ST = S // 128   # 16 seq tiles
NQ = S // 512   # 4 query chunks of 512


def _ins_bcast(ap, pos, n):
    """Insert a stride-0 (broadcast) dim of size n at free-dim position pos."""
    a = [list(p) for p in ap.ap]
    return bass.AP(tensor=ap.tensor, offset=ap.offset,
                   ap=a[:pos] + [[0, n]] + a[pos:])


def _bcast_ap(src_ap, parts):
    """Partition-broadcast view: repeat src_ap's single row across `parts`."""
    ap = [list(p) for p in src_ap.ap]
    if len(ap) > 1 and ap[0][1] == 1:
        ap = ap[1:]  # drop singleton partition dim
    return bass.AP(
        tensor=src_ap.tensor,
        offset=src_ap.offset,
        ap=[[0, parts]] + ap,
    )


def build_nc():
    nc = bacc.Bacc(None, target_bir_lowering=False, debug=False, num_devices=NCORES)

    xb = nc.declare_dram_parameter("xb", [S, H], BF16, isOutput=False)
    wq = nc.declare_dram_parameter("wq", [H, CW], BF16, isOutput=False)
    wk = nc.declare_dram_parameter("wk", [H, CW], BF16, isOutput=False)
    wv = nc.declare_dram_parameter("wv", [H, CW], BF16, isOutput=False)
    wp = nc.declare_dram_parameter("wp", [H, H], BF16, isOutput=False)
    qoff = nc.declare_dram_parameter("qoff", [1, 1], mybir.dt.int32, isOutput=False)
    bq = nc.declare_dram_parameter("bq", [CW], F32, isOutput=False)
    bk = nc.declare_dram_parameter("bk", [CW], F32, isOutput=False)
    bv = nc.declare_dram_parameter("bv", [CW], F32, isOutput=False)
    out = nc.declare_dram_parameter("out", [SLICE, H], F32, isOutput=True)

    with tile.TileContext(nc) as tc:
        with (
            tc.tile_pool(name="dram", bufs=1, space="DRAM") as dram,
            tc.tile_pool(name="psum", bufs=1, space="PSUM") as psum,
            tc.tile_pool(name="persist", bufs=1) as pw,
        ):
            ag_in = dram.tile([NQ, CW, 512], BF16)   # my heads' O^T, per chunk
            gathered = dram.tile([NQ * 4 * CW, 512], BF16)  # [chunk, all heads]

            ident = pw.tile([128, 128], BF16)
            ident_f32 = pw.tile([128, 128], F32)
            make_identity(nc, ident_f32)
            nc.vector.tensor_copy(ident, ident_f32)
            # lower-triangle-in-q mask: tri[k, q] = 1 if q >= k else 0
            tri_f32 = pw.tile([128, 128], F32)
            nc.gpsimd.memset(tri_f32, 1.0)
            nc.gpsimd.affine_select(
                out=tri_f32, in_=tri_f32, compare_op=mybir.AluOpType.is_ge,
                fill=0.0, base=0, pattern=[[1, 128]], channel_multiplier=-1)
            tri = pw.tile([128, 128], BF16)
            nc.vector.tensor_copy(tri, tri_f32)


            # qkv weights: [128, k-tile, cols]
            wq_sb = pw.tile([128, KT, CW], BF16)
            wk_sb = pw.tile([128, KT, CW], BF16)
            wv_sb = pw.tile([128, KT, CW], BF16)
            nc.gpsimd.dma_start(out=wq_sb, in_=wq.ap().rearrange("(k p) c -> p k c", p=128))
            nc.gpsimd.dma_start(out=wk_sb, in_=wk.ap().rearrange("(k p) c -> p k c", p=128))
            nc.gpsimd.dma_start(out=wv_sb, in_=wv.ap().rearrange("(k p) c -> p k c", p=128))
            wp_sb = pw.tile([128, KT, H], BF16)  # loaded after QKV (below)

            # biases
            bq_sb = pw.tile([128, 2], F32)
            bk_sb = pw.tile([128, 2], F32)
            nc.gpsimd.dma_start(out=bq_sb, in_=bq.ap().rearrange("(h p) -> p h", p=128))
            nc.gpsimd.dma_start(out=bk_sb, in_=bk.ap().rearrange("(h p) -> p h", p=128))
            bv_bc = pw.tile([128, CW], F32)
            nc.gpsimd.dma_start(out=bv_bc, in_=_bcast_ap(bv.ap(), 128))

            QTt = pw.tile([128, 2, S], BF16)   # q-col (128) x [half, s]
            KTt = pw.tile([128, 2, S], BF16)
            V4 = pw.tile([128, ST, HPC, DK + 1], BF16)  # [s-part, s-tile, head, dk|1]
            nc.gpsimd.memset(V4, 1.0)  # ones column (rest overwritten)

            with tc.tile_pool(name="px", bufs=1) as px:
                xT = px.tile([128, KT, S], BF16)   # h-part x [h-tile, s]

                # ---- phases 0+1 fused per chunk: x^T then that chunk's QKV
                cp_engines = [nc.vector.tensor_copy, nc.vector.tensor_copy,
                              nc.scalar.copy]
                nco = 0
                for j in range(NQ):
                    for si in range(4 * j, 4 * j + 4):
                        xs = px.tile([128, H], BF16, tag="xs", bufs=4)
                        nc.sync.dma_start(out=xs,
                                          in_=xb[si * 128:(si + 1) * 128, :])
                        for kh in range(2):
                            ptp = psum.tile([128, 512], BF16, tag="p512", bufs=2)
                            for i in range(4):
                                k = 4 * kh + i
                                nc.tensor.transpose(
                                    ptp[:, i * 128:(i + 1) * 128],
                                    xs[:, k * 128:(k + 1) * 128], ident)
                            src = ptp[:, :].rearrange("p (k f) -> p k f", k=4)
                            dst = xT[:, 4 * kh:4 * kh + 4,
                                     si * 128:(si + 1) * 128]
                            cp_engines[nco % 3](dst, src)
                            nco += 1
                    js = slice(j * 512, (j + 1) * 512)
                    for ti, (wt, dst, bias) in enumerate(
                            ((wq_sb, QTt, bq_sb), (wk_sb, KTt, bk_sb))):
                        for half in range(2):
                            pq = psum.tile([128, 512], F32, tag="p512", bufs=2)
                            for k in range(KT):
                                nc.tensor.matmul(
                                    pq,
                                    wt[:, k, half * 128:(half + 1) * 128],
                                    xT[:, k, js],
                                    start=(k == 0),
                                    stop=(k == KT - 1),
                                )
                            if ti == 0:
                                nc.scalar.activation(
                                    dst[:, half, js], pq,
                                    mybir.ActivationFunctionType.Identity,
                                    bias=bias[:, half:half + 1],
                                )
                            else:
                                nc.vector.tensor_scalar_add(
                                    dst[:, half, js], pq, bias[:, half:half + 1])
                    for pi in range(2):
                        si0 = 4 * j + 2 * pi
                        pv = psum.tile([128, 512], F32, tag="p512", bufs=2)
                        for i in range(2):
                            for k in range(KT):
                                nc.tensor.matmul(
                                    pv[:, 256 * i:256 * (i + 1)],
                                    xT[:, k, (si0 + i) * 128:(si0 + i + 1) * 128],
                                    wv_sb[:, k, :],
                                    start=(k == 0), stop=(k == KT - 1),
                                )
                        pv_v = pv[:, :].rearrange("p (s h d) -> p s h d", s=2, h=HPC)
                        bv_h = bv_bc[:, :].rearrange("p (h d) -> p h d", h=HPC)
                        bv_v = _ins_bcast(bv_h, 1, 2)
                        nc.vector.tensor_add(V4[:, si0:si0 + 2, :, 0:DK],
                                             pv_v, bv_v)

            # c_proj weights: deferred here so the load doesn't contend with
            # the x/QKV-weight DMAs at kernel start
            nc.gpsimd.dma_start(
                out=wp_sb, in_=wp.ap().rearrange("(k p) c -> p k c", p=128))

            # ---- phase 2: attention j-outer / h-inner + chunk AllGathers ----
            with tc.tile_pool(name="pa", bufs=1) as pa:
                sums4 = [dram.tile([HPC, 512], BF16, name=f"sums4_{j}")
                         for j in range(NQ)]
                rsums4 = [dram.tile([HPC, 512], BF16, name=f"rsums4_{j}")
                          for j in range(NQ)]
                for j in range(NQ):
                    combs = []
                    for h in range(HPC):
                        pb = 64 * (h % 2)
                        qt = QTt[pb:pb + DK, h // 2, :]
                        kt = KTt[pb:pb + DK, h // 2, :]
                        nblk = 4 * j + 4
                        ngrp = nblk // 2
                        pav = psum.tile([65, 512], F32, tag="p512", bufs=2,
                                        name=f"pav{h}_{j}")
                        Ags = {}

                        def do_av(g):
                            Ag = Ags.pop(g)
                            for i in range(2):
                                ki = 2 * g + i
                                off = max(0, 128 * ki - 512 * j)
                                npp = 512 - off
                                nc.tensor.matmul(
                                    pav[:, off:], V4[:, ki, h, :],
                                    Ag[:, 512 * i:512 * i + npp],
                                    start=(ki == 0), stop=(ki == nblk - 1),
                                )

                        for g in range(ngrp):
                            sg = psum.tile([128, 1024], F32, tag="sg", bufs=3,
                                           name=f"sg{h}_{j}_{g}")
                            Ag = pa.tile([128, 1024], BF16, tag="A", bufs=4,
                                         name=f"A{h}_{j}_{g}")
                            ext = 0
                            for i in range(2):
                                ki = 2 * g + i
                                off = max(0, 128 * ki - 512 * j)
                                npp = 512 - off
                                nc.tensor.matmul(
                                    sg[:, 512 * i:512 * i + npp],
                                    kt[:, ki * 128:(ki + 1) * 128],
                                    qt[:, j * 512 + off:(j + 1) * 512],
                                    start=True, stop=True,
                                )
                                ext = 512 * i + npp
                            nc.scalar.activation(
                                Ag[:, :ext], sg[:, :ext],
                                mybir.ActivationFunctionType.Exp)
                            if 2 * g >= 4 * j:
                                # batched causal corner mask over both blocks
                                av = Ag[:, :].rearrange(
                                    "p (b f) -> p b f", b=2)[:, :, 0:128]
                                nc.vector.tensor_mul(
                                    av, av, _ins_bcast(tri[:, :], 1, 2))
                            Ags[g] = Ag
                            if g >= 1:
                                do_av(g - 1)
                        do_av(ngrp - 1)

                        # PSUM row 64 = softmax denominators: stage the raw
                        # sums row into this chunk's [4, 512] DRAM tile; the
                        # reciprocal runs once per chunk (batched, below).
                        comb = pa.tile([65, 512], BF16, tag="comb", bufs=5,
                                       name=f"comb{h}_{j}")
                        nc.vector.tensor_copy(comb, pav)
                        combs.append(comb)
                        nc.sync.dma_start(out=sums4[j][h, :], in_=comb[64:65, :])

                    # batched per-chunk normalization: one [128, 4, 4] DVE
                    # reciprocal (a [1,512] DVE recip costs ~8 cyc/free-elem;
                    # ACT recip would thrash the exp table set), one bounce
                    # back, then per-head broadcast + multiply on GPSIMD so
                    # no DMA latency ever blocks the DVE FIFO (tri-masks).
                    sre = pa.tile([128, 4, 4], BF16, tag="sre", bufs=2,
                                  name=f"sre{j}")
                    nc.sync.dma_start(
                        out=sre,
                        in_=sums4[j].rearrange("h (p f) -> p h f", p=128))
                    with nc.allow_low_precision(
                            reason="bf16 recip of O(1e3) softmax sums"):
                        nc.vector.reciprocal(sre, sre)
                    nc.sync.dma_start(
                        out=rsums4[j].rearrange("h (p f) -> p h f", p=128),
                        in_=sre)
                    for h in range(HPC):
                        rbc = pa.tile([64, 512], BF16, tag="rbc", bufs=4,
                                      name=f"rbc{h}_{j}")
                        nc.gpsimd.dma_start(
                            out=rbc, in_=_bcast_ap(rsums4[j][h, :], 64))
                        combn = pa.tile([64, 512], BF16, tag="combn", bufs=4,
                                        name=f"combn{h}_{j}")
                        nc.gpsimd.tensor_mul(combn, combs[h][0:64, :], rbc)
                        # pack into this chunk's AllGather input
                        nc.sync.dma_start(
                            out=ag_in[j, 64 * h:64 * h + 64, :],
                            in_=combn)
                    nc.gpsimd.collective_compute(
                        "AllGather",
                        mybir.AluOpType.bypass,
                        replica_groups=GROUPS,
                        ins=[ag_in[j].opt()],
                        outs=[gathered[1024 * j:1024 * (j + 1), :].opt()],
                    )

                # HAM warm-keeper: dummy matmuls that execute only while the
                # PE would otherwise idle during the last chunk's
                # normalization chain + AllGather (~40us), so c_proj starts
                # at full clock instead of 1.2GHz. Results are never read.
                for wi in range(64):
                    pwm = psum.tile([128, 512], F32, tag="p512", bufs=2,
                                    name=f"pwm{wi}")
                    nc.tensor.matmul(
                        pwm, wp_sb[:, wi % KT, 0:128],
                        wp_sb[:, (wi + 1) % KT, 0:512],
                        start=True, stop=True)

                # ---- c_proj of my 512-row slice, full head dimension ----
                qoff_sb = pa.tile([1, 1], mybir.dt.int32)
                nc.gpsimd.dma_start(out=qoff_sb, in_=qoff[:, :])
                og_sb = pa.tile([128, KT, SLICE], BF16)
                with nc.gpsimd.register("qor") as qor:
                    nc.gpsimd.load(qor, qoff_sb[0:1, 0:1])
                    qsnap = nc.gpsimd.snap(qor)
                    nc.gpsimd.dma_start(
                        out=og_sb,
                        in_=gathered[bass.ds(qsnap, 4 * CW), :].rearrange(
                            "(k p) q -> p k q", p=128))
                for t in range(4):
                    pt = psum.tile([128, 1024], F32, tag="sg", bufs=3,
                                   name=f"pt{t}")
                    for n in range(2):
                        for k in range(KT):
                            nc.tensor.matmul(
                                pt[:, n * 512:(n + 1) * 512],
                                og_sb[:, k, t * 128:(t + 1) * 128],
                                wp_sb[:, k, n * 512:(n + 1) * 512],
                                start=(k == 0), stop=(k == KT - 1),
                            )
                    yt = pa.tile([128, H], F32, tag="yt", bufs=2, name=f"yt{t}")
                    if t % 2 == 0:
                        nc.vector.tensor_copy(yt, pt)
                    else:
                        nc.scalar.copy(yt, pt)
                    nc.sync.dma_start(out=out[t * 128:(t + 1) * 128, :], in_=yt)

    nc.compile()
    return nc


def make_in_maps(x, w_attn, b_attn, w_proj):
    import ml_dtypes
    bf = ml_dtypes.bfloat16
    x = np.asarray(x, dtype=np.float32)
    w_attn = np.asarray(w_attn, dtype=np.float32)
    b_attn = np.asarray(b_attn, dtype=np.float32)
    w_proj = np.asarray(w_proj, dtype=np.float32)
    wp_bf = np.ascontiguousarray(w_proj).astype(bf)
    in_maps = []
    for c in range(NCORES):
        b, g = divmod(c, 4)
        h0 = g * HPC
        cs = slice(h0 * DK, h0 * DK + CW)
        in_maps.append({
            "xb": np.ascontiguousarray(x[b]).astype(bf),
            # fold the 1/sqrt(DK)=2^-3 score scale into Wq/bq (exact in fp32)
            "wq": (np.ascontiguousarray(w_attn[:, cs])
                   * np.float32(0.125)).astype(bf),
            "wk": np.ascontiguousarray(w_attn[:, H:][:, cs]).astype(bf),
            "wv": np.ascontiguousarray(w_attn[:, 2 * H:][:, cs]).astype(bf),
            "wp": wp_bf,
            "qoff": np.array([[g * 4 * CW]], dtype=np.int32),
            "bq": np.ascontiguousarray(b_attn[cs]) * np.float32(0.125),
            "bk": np.ascontiguousarray(b_attn[H:][cs]),
            "bv": np.ascontiguousarray(b_attn[2 * H:][cs]),
        })
    return in_maps


_NC = None


def kernel(x, w_attn, b_attn, w_proj, b_proj):
    global _NC
    if _NC is None:
        _NC = build_nc()

    b_proj = np.asarray(b_proj, dtype=np.float32)
    in_maps = make_in_maps(x, w_attn, b_attn, w_proj)
    res = run_bass_kernel_spmd(_NC, in_maps, core_ids=list(range(NCORES)))

    outp = np.empty((B, S, H), dtype=np.float32)
    for c in range(NCORES):
        b, g = divmod(c, 4)
        outp[b, g * SLICE:(g + 1) * SLICE, :] = res.results[c]["out"]
    outp += b_proj  # row-broadcast add, exact
    return outp
